# revision 50
# baseline (speedup 1.0000x reference)
"""AttentionMIL pooling kernel for 8 Trainium2 NeuronCores.

Math (per slide b): h = tanh(X @ W1^T); s = h @ w2; a = softmax(s);
out = a^T @ X, with X [N=8192, D=1024], W1 [H=256, D], w2 [H].

Strategy (v8 — pair-grouped wsum with head singles):
  - Data-parallel over the slide dim: 16 slides / 8 cores = 2 per core.
  - ONE host-swizzled transposed copy of X per core (bf16, [128(d-chunk),
    n-free] tiles) — 32 MiB of HBM traffic per core (DMA floor ~94us).
  - h^T per 512-row n-tile on PE (16 MMs, k-outer), tanh on ACT, scores
    via REPLICATED-w2 stationary (broadcast score row; no max pass since
    |s| <= ||w2||_1 keeps exp in fp32 range), exp on ACT.
  - The weighted sum runs on DVE+ACT at 1x (fused STT / copy-accum), so
    per-op FIXED cost (~150-500ns) is the trimmable part: n-tiles are
    processed in GROUPS (two head singles, then pairs) and every wsum op
    spans the whole group: one xt tile per group (DMA'd in 1 MiB pieces
    so h never waits), both score rows in one PSUM tile, ONE exp over
    G*512 cols, DVE STT with accum_out and ACT copy-accum over (G,512)
    ranges -> per-group partials.  The head singles' score/exp/wsum are
    emitted INLINE (PE is DMA-paced there, score MMs ride its stall
    gaps) which starts DVE ~8us earlier.  The final pair runs a
    4-ACT/4-DVE chunk split with per-chunk muls so the drain stays
    balanced (ACT's extra chunk lands in racc_d's column).
    Quad groups were tried and are WORSE: a group's wsum needs its last
    tile's exp, so 4-tile groups lump e-delivery and stall DVE.
  - Steady split: chunks 0-2 -> one 2x DVE mul + ACT copy-accum; chunks
    3-7 -> DVE fused STT.  Partials land in disjoint columns of
    per-engine zero-initialised accumulators (a shared tile would thread
    false cross-engine deps); one final reduce folds 8 group-columns.
  - GpSimd stays out of the hot loop (its SBUF port is physically shared
    with DVE; tensor ops there poison DVE ~2.5x, measured).
  - l = sum(e): group score rows DMA'd out (sync queue, emitted two
    groups late so their waits never block xt dispatches), summed on
    host; out = acc / l on host.
  - Startup: w1t halves first on the sync HWDGE queue interleaved with
    quarter DMAs of tile 0; 34 128-col warm matmuls bridge the preamble
    (HAM re-throttles after >3.4us PE idle; cold MMs run at half rate).
  - Pipeline: the previous group's last score MMs + exp ride in the
    MIDDLE of the next group's first h-block (k==4), so e(prev) exists
    ~2us into the group and DVE's wsum follows in the same iteration.
"""

import sys

sys.path.insert(0, "/opt/trn_rl_repo")

import numpy as np
import ml_dtypes

import concourse.bacc as bacc
import concourse.tile as tile
from concourse import mybir
from concourse.bass_utils import run_bass_kernel_spmd

BF16 = ml_dtypes.bfloat16
B, N, D, H = 16, 8192, 1024, 256
NCORES = 8
SPC = B // NCORES          # slides per core
NT = 512                   # rows of N per tile
TILES = N // NT
KCH = D // 128             # d-chunks (contraction chunks of 128)
HH = H // 128              # h halves
GP = 3                     # d-chunks routed DVE(mul) + Scalar(accum-reduce)
GP_LAST = 4                # ...on the final group (drain balance)
GMAX = 2                   # largest group
NG = 10                    # racc columns per chunk (>= groups per slide)
WARM_MMS = 34
OUTE_LAG = 2
# group sizes per slide: pairs amortize per-op fixed costs without the
# pipeline-fill bubble quads showed (e for a group needs all its
# h-blocks); two head singles fill the DVE/ACT pipeline fast
GROUP_SCHED = {0: [1, 1] + [2] * 7, SPC - 1: [2] * 7 + [1, 1]}
GROUP_DEFAULT = [2] * 8

_NC_CACHE = {}


def _build_nc():
    bf = mybir.dt.bfloat16
    f32 = mybir.dt.float32
    AF = mybir.ActivationFunctionType
    OP = mybir.AluOpType

    nc = bacc.Bacc("TRN2", num_devices=NCORES)
    # Host-swizzled transposed layout, tile-major: per tile one contiguous
    # 1 MiB region.  xt[s, t, q, k*NT + j] = X[s, t*NT + j, k*128 + q]
    xt = nc.declare_dram_parameter("xt", [SPC, TILES, 128, KCH * NT], bf, isOutput=False)
    # w1t[q, k*H + h] = W1[h, k*128 + q]
    w1t = nc.declare_dram_parameter("w1t", [128, KCH * H], bf, isOutput=False)
    # w2rep[q, half*128 + c] = W2[0, half*128 + q]  (replicated along c)
    w2rep = nc.declare_dram_parameter("w2rep", [128, H], bf, isOutput=False)
    outp = nc.declare_dram_parameter("out", [SPC, 128, KCH], f32, isOutput=True)
    # e rows, summed on host for the softmax denominator (flat per slide)
    oute = nc.declare_dram_parameter("oute", [SPC, TILES * NT], bf, isOutput=True)

    with tile.TileContext(nc) as tc:
        with tc.tile_pool(name="const", bufs=1) as constp, \
             tc.tile_pool(name="xt", bufs=4) as xtp, \
             tc.tile_pool(name="tanh", bufs=3) as tp, \
             tc.tile_pool(name="e128", bufs=4) as ep, \
             tc.tile_pool(name="scr", bufs=2) as scrp, \
             tc.tile_pool(name="scra", bufs=2) as scrap, \
             tc.tile_pool(name="gprod", bufs=3) as gprodp, \
             tc.tile_pool(name="racc", bufs=2) as raccp, \
             tc.tile_pool(name="outsb", bufs=2) as outsbp, \
             tc.tile_pool(name="hps", bufs=2, space="PSUM") as hpsp, \
             tc.tile_pool(name="sps", bufs=1, space="PSUM") as spsp:

            # w1t gates every h-matmul: its first half precedes everything
            # in the HWDGE FIFO; second half rides between xt quarters.
            w1t_sb = constp.tile([128, KCH * H], bf)
            nc.sync.dma_start(w1t_sb[:, 0:KCH * H // 2], w1t[:, 0:KCH * H // 2])
            w2r_sb = constp.tile([128, H], bf)

            warm_sb = constp.tile([128, 256], bf)
            # memset on DVE: its preamble ends earliest, warm MMs start ASAP
            nc.vector.memset(warm_sb[:], 0.0)
            warm_ps = hpsp.tile([128, 256], f32, tag="hps")
            for _ in range(WARM_MMS):
                nc.tensor.matmul(
                    warm_ps[:, 0:128], warm_sb[:, 0:128], warm_sb[:, 0:128],
                    start=True, stop=True, skip_group_check=True,
                )

            state = {}          # per-slide persistent accumulator tiles
            pend_oute = []      # [(s, t0, G, e_g)] stores, emitted late

            def h_mms(xt_g, u, h_ps, mid=None):
                # k-outer so each arriving xt piece unblocks its MMs; `mid`
                # emits the previous tile's score MMs (and possibly the
                # previous group's exp) halfway through the block.
                for k in range(KCH):
                    if k == KCH // 2 and mid is not None:
                        mid()
                    for half in range(HH):
                        nc.tensor.matmul(
                            h_ps[:, half * NT:(half + 1) * NT],
                            w1t_sb[:, k * H + half * 128: k * H + half * 128 + 128],
                            xt_g[:, (u * KCH + k) * NT:(u * KCH + k + 1) * NT],
                            start=(k == 0), stop=(k == KCH - 1),
                        )

            def score_mms(s_ps, u, tanh_sb):
                for half in range(HH):
                    nc.tensor.matmul(
                        s_ps[:, u * NT:(u + 1) * NT],
                        w2r_sb[:, half * 128:(half + 1) * 128],
                        tanh_sb[:, half * NT:(half + 1) * NT],
                        start=(half == 0), stop=(half == HH - 1),
                        skip_group_check=True,
                    )

            def emit_exp(s, t0, G, gi, s_ps, xt_g):
                e_g = ep.tile([128, GMAX * NT], bf, tag="e")
                nc.scalar.activation(e_g[:, 0:G * NT], s_ps[:, 0:G * NT], AF.Exp)
                pend_oute.append((s, t0, G, e_g))
                return (s, gi, G, xt_g, e_g, t0)

            def emit_wsum(s, gi, G, xt_g, e_g, t0):
                racc_a, racc_d = state[s]
                last = t0 + G == TILES and s == SPC - 1
                gp = GP_LAST if last else GP
                xt4 = xt_g[:, 0:G * KCH * NT].rearrange(
                    "q (u k j) -> q u k j", u=G, k=KCH
                )
                e3 = e_g[:, 0:G * NT].rearrange("q (u j) -> q u j", u=G)
                # products for the ACT-owned chunks first (2x-rate, one op)
                # so the Scalar engine's accum copies unblock early
                prod = gprodp.tile(
                    [128, max(GMAX * GP, 2 * GP_LAST) * NT], bf, tag="prod",
                )
                prod4 = prod[:, 0:G * gp * NT].rearrange(
                    "q (u k j) -> q u k j", u=G, k=gp
                )
                if last:
                    # drain: per-chunk muls so the Scalar engine's first
                    # accum copy starts after ~0.6us instead of the full mul
                    for k in range(gp):
                        nc.vector.tensor_mul(
                            prod4[:, :, k:k + 1, :],
                            xt4[:, :, k:k + 1, :],
                            e3[:, :, None, :].broadcast_to([128, G, 1, NT]),
                        )
                else:
                    nc.vector.tensor_mul(
                        prod4[:, :, 0:gp, :],
                        xt4[:, :, 0:gp, :],
                        e3[:, :, None, :].broadcast_to([128, G, gp, NT]),
                    )
                scr = scrp.tile([128, GMAX * NT], bf, tag="scr")
                for k in range(gp, KCH):
                    nc.vector.scalar_tensor_tensor(
                        scr[:, 0:G * NT].rearrange("q (u j) -> q u j", u=G),
                        xt4[:, :, k, :],
                        1.0,
                        e3[:],
                        op0=OP.mult,
                        op1=OP.mult,
                        accum_out=racc_d[:, (k - GP) * NG + gi:
                                         (k - GP) * NG + gi + 1],
                    )
                return (s, gi, G, gp, prod4, t0)

            def emit_accums(s, gi, G, gp, prod4, t0):
                racc_a, racc_d = state[s]
                scr_a = scrap.tile([128, GMAX * NT], bf, tag="scra")
                for k in range(gp):
                    # the drain group's extra ACT chunks land in racc_d's
                    # column for that (chunk, group) — zeros elsewhere
                    acc = (
                        racc_a[:, k * NG + gi: k * NG + gi + 1]
                        if k < GP
                        else racc_d[:, (k - GP) * NG + gi:
                                    (k - GP) * NG + gi + 1]
                    )
                    nc.scalar.activation(
                        scr_a[:, 0:G * NT].rearrange("q (u j) -> q u j", u=G),
                        prod4[:, :, k, :],
                        AF.Copy,
                        accum_out=acc,
                    )
                if t0 + G == TILES:
                    out_sb = outsbp.tile([128, KCH], f32)
                    nc.vector.reduce_sum(
                        out_sb[:, 0:GP],
                        racc_a[:].rearrange("q (k g) -> q k g", k=GP),
                        axis=mybir.AxisListType.X,
                    )
                    nc.vector.reduce_sum(
                        out_sb[:, GP:KCH],
                        racc_d[:].rearrange("q (k g) -> q k g", k=KCH - GP),
                        axis=mybir.AxisListType.X,
                    )
                    nc.gpsimd.dma_start(outp[s], out_sb[:])

            def flush_oute(keep):
                while len(pend_oute) > keep:
                    s_, t0_, G_, e_ = pend_oute.pop(0)
                    nc.sync.dma_start(
                        oute[s_:s_ + 1, t0_ * NT:(t0_ + G_) * NT],
                        e_[0:1, 0:G_ * NT],
                    )

            groups = []
            for s in range(SPC):
                t0 = 0
                for G in GROUP_SCHED.get(s, GROUP_DEFAULT):
                    groups.append((s, t0, G))
                    t0 += G
                assert t0 == TILES

            pend_score = None   # (s_ps, u, tanh) last tile of prev group
            cur_group = None    # (s, t0, G, gi, s_ps, xt_g) being scored
            first = True

            for s, t0, G in groups:
                if t0 == 0:
                    gi = 0
                    # separate accumulators for the ACT and DVE chunk sets —
                    # a shared tile would thread false cross-engine deps.
                    # Zero-filled: unused group-columns must fold as 0.
                    racc_a = raccp.tile(
                        [128, GP * NG], f32, tag=f"racca{s}", name=f"racca{s}",
                    )
                    racc_d = raccp.tile(
                        [128, (KCH - GP) * NG], f32,
                        tag=f"raccd{s}", name=f"raccd{s}",
                    )
                    nc.vector.memset(racc_a[:], 0.0)
                    nc.vector.memset(racc_d[:], 0.0)
                    state[s] = (racc_a, racc_d)
                xt_g = xtp.tile([128, GMAX * KCH * NT], bf, tag="xt")
                for u in range(G):
                    dst = xt_g[:, u * KCH * NT:(u + 1) * KCH * NT]
                    src = xt[s, t0 + u]
                    if first and u == 0:
                        # quarters interleaved with w1t's second half and
                        # w2rep: first h-MMs start after ~1.3 MiB
                        nc.sync.dma_start(dst[:, 0:2 * NT], src[:, 0:2 * NT])
                        nc.sync.dma_start(
                            w1t_sb[:, KCH * H // 2:], w1t[:, KCH * H // 2:],
                        )
                        for q in range(1, 4):
                            nc.sync.dma_start(
                                dst[:, q * 2 * NT:(q + 1) * 2 * NT],
                                src[:, q * 2 * NT:(q + 1) * 2 * NT],
                            )
                            if q == 1:
                                nc.sync.dma_start(w2r_sb[:], w2rep[:, :])
                    else:
                        nc.sync.dma_start(dst, src)

                s_ps = spsp.tile([128, GMAX * NT], f32, tag="sps")
                exh = [None]

                def mid0():
                    # previous group's last score MMs + its exp: exp
                    # precedes this group's tanhs in the ACT FIFO so DVE's
                    # wsum input is ready ~2us into the group
                    if pend_score is not None:
                        ps_, pu_, ptanh = pend_score
                        score_mms(ps_, pu_, ptanh)
                        exh[0] = emit_exp(*cur_group)

                tanh_prev = None
                for u in range(G):
                    h_ps = hpsp.tile(
                        [128, HH * NT], f32, name=f"h_ps{u}", tag="hps",
                    )
                    if u == 0:
                        h_mms(xt_g, 0, h_ps, mid=mid0)
                    else:
                        tp_prev = tanh_prev
                        uu = u - 1
                        h_mms(
                            xt_g, u, h_ps,
                            mid=lambda: score_mms(s_ps, uu, tp_prev),
                        )
                    tanh_u = tp.tile([128, HH * NT], bf, tag="tanh")
                    t_glob = t0 + u
                    if (s == 0 and t_glob < 2) or (
                        s == SPC - 1 and t_glob == TILES - 1
                    ):
                        # head singles + the drain tile: tanh in halves so
                        # the immediately-following score MMs (and exp)
                        # start half a tanh earlier
                        for hh in range(HH):
                            nc.scalar.activation(
                                tanh_u[:, hh * NT:(hh + 1) * NT],
                                h_ps[:, hh * NT:(hh + 1) * NT],
                                AF.Tanh,
                            )
                    else:
                        nc.scalar.activation(tanh_u[:], h_ps[:], AF.Tanh)
                    tanh_prev = tanh_u
                    if u == min(1, G - 1) and exh[0] is not None:
                        # wsum + accums for the previous group; accum
                        # copies sit after this group's early tanhs in the
                        # ACT FIFO so the PE score path never waits
                        wa = emit_wsum(*exh[0])
                        emit_accums(*wa)
                        exh[0] = None

                if G == 1 and s == 0 and t0 < 2:
                    # head singles: PE is DMA-paced here, so their score
                    # MMs ride in its stall gaps — score+exp+wsum inline,
                    # which starts DVE ~8us earlier than mid-next placement
                    score_mms(s_ps, 0, tanh_prev)
                    ex0 = emit_exp(s, t0, G, gi, s_ps, xt_g)
                    wa = emit_wsum(*ex0)
                    emit_accums(*wa)
                    pend_score = None
                else:
                    pend_score = (s_ps, G - 1, tanh_prev)
                    cur_group = (s, t0, G, gi, s_ps, xt_g)
                gi += 1
                first = False
                flush_oute(OUTE_LAG)

            # drain: score+exp+wsum for the final group only
            ps_, pu_, ptanh = pend_score
            score_mms(ps_, pu_, ptanh)
            ex = emit_exp(*cur_group)
            wa = emit_wsum(*ex)
            emit_accums(*wa)
            flush_oute(0)

    nc.compile()
    return nc


def _get_nc():
    if "nc" not in _NC_CACHE:
        _NC_CACHE["nc"] = _build_nc()
    return _NC_CACHE["nc"]


def _prep_inputs(tiles_embeddings, W1, W2):
    X_bf = tiles_embeddings.astype(BF16)
    # xt[b, t, q, k, j] = X[b, t*NT + j, k*128 + q]
    xt_sw = np.ascontiguousarray(
        X_bf.reshape(B, TILES, NT, KCH, 128).transpose(0, 1, 4, 3, 2)
    ).reshape(B, TILES, 128, KCH * NT)
    # w1t[q, k, h] = W1[h, k*128 + q]
    w1t = np.ascontiguousarray(
        W1.astype(BF16).reshape(H, KCH, 128).transpose(2, 1, 0)
    ).reshape(128, KCH * H)
    # w2rep[q, half*128 + c] = W2[0, half*128 + q]
    w2rep = np.ascontiguousarray(
        np.broadcast_to(
            W2.astype(BF16).reshape(HH, 128).transpose(1, 0)[:, :, None],
            (128, HH, 128),
        )
    ).reshape(128, H)
    return [
        {
            "xt": xt_sw[c * SPC:(c + 1) * SPC],
            "w1t": w1t,
            "w2rep": w2rep,
        }
        for c in range(NCORES)
    ]


def _run(tiles_embeddings, W1, W2, **spmd_kwargs):
    nc = _get_nc()
    in_maps = _prep_inputs(tiles_embeddings, W1, W2)
    res = run_bass_kernel_spmd(nc, in_maps, core_ids=list(range(NCORES)), **spmd_kwargs)
    acc = np.concatenate([r["out"] for r in res.results], axis=0)       # [B, 128, KCH]
    e = np.concatenate([r["oute"] for r in res.results], axis=0)        # [B, TILES*NT]
    l = e.astype(np.float64).sum(axis=1)                                # [B]
    # out[b, k*128 + q] = acc[b, q, k]
    out = acc.transpose(0, 2, 1).reshape(B, D) / l[:, None]
    return out.astype(np.float32, copy=False), res


def kernel(tiles_embeddings, W1, W2):
    out, _ = _run(
        np.asarray(tiles_embeddings), np.asarray(W1), np.asarray(W2)
    )
    return out


# revision 53
# speedup vs baseline: 1.0073x; 1.0073x over previous
"""AttentionMIL pooling kernel for 8 Trainium2 NeuronCores.

Math (per slide b): h = tanh(X @ W1^T); s = h @ w2; a = softmax(s);
out = a^T @ X, with X [N=8192, D=1024], W1 [H=256, D], w2 [H].

Strategy (v8 — pair-grouped wsum with head singles):
  - Data-parallel over the slide dim: 16 slides / 8 cores = 2 per core.
  - ONE host-swizzled transposed copy of X per core (bf16, [128(d-chunk),
    n-free] tiles) — 32 MiB of HBM traffic per core (DMA floor ~94us).
  - h^T per 512-row n-tile on PE (16 MMs, k-outer), tanh on ACT, scores
    via REPLICATED-w2 stationary (broadcast score row; no max pass since
    |s| <= ||w2||_1 keeps exp in fp32 range), exp on ACT.
  - The weighted sum runs on DVE+ACT at 1x (fused STT / copy-accum), so
    per-op FIXED cost (~150-500ns) is the trimmable part: n-tiles are
    processed in GROUPS (two head singles, then pairs) and every wsum op
    spans the whole group: one xt tile per group (DMA'd in 1 MiB pieces
    so h never waits), both score rows in one PSUM tile, ONE exp over
    G*512 cols, DVE STT with accum_out and ACT copy-accum over (G,512)
    ranges -> per-group partials.  The head singles' score/exp/wsum are
    emitted INLINE (PE is DMA-paced there, score MMs ride its stall
    gaps) which starts DVE ~8us earlier.  The final pair runs a
    4-ACT/4-DVE chunk split with per-chunk muls so the drain stays
    balanced (ACT's extra chunk lands in racc_d's column).
    Quad groups were tried and are WORSE: a group's wsum needs its last
    tile's exp, so 4-tile groups lump e-delivery and stall DVE.
  - Steady split: chunks 0-2 -> one 2x DVE mul + ACT copy-accum; chunks
    3-7 -> DVE fused STT.  Partials land in disjoint columns of
    per-engine zero-initialised accumulators (a shared tile would thread
    false cross-engine deps); one final reduce folds 8 group-columns.
  - GpSimd stays out of the hot loop (its SBUF port is physically shared
    with DVE; tensor ops there poison DVE ~2.5x, measured).
  - l = sum(e): group score rows DMA'd out (sync queue, emitted two
    groups late so their waits never block xt dispatches), summed on
    host; out = acc / l on host.
  - Startup: w1t halves first on the sync HWDGE queue interleaved with
    quarter DMAs of tile 0; 34 128-col warm matmuls bridge the preamble
    (HAM re-throttles after >3.4us PE idle; cold MMs run at half rate).
  - Pipeline: the previous group's last score MMs + exp ride in the
    MIDDLE of the next group's first h-block (k==4), so e(prev) exists
    ~2us into the group and DVE's wsum follows in the same iteration.
"""

import sys

sys.path.insert(0, "/opt/trn_rl_repo")

import numpy as np
import ml_dtypes

import concourse.bacc as bacc
import concourse.tile as tile
from concourse import mybir
from concourse.bass_utils import run_bass_kernel_spmd

BF16 = ml_dtypes.bfloat16
B, N, D, H = 16, 8192, 1024, 256
NCORES = 8
SPC = B // NCORES          # slides per core
NT = 512                   # rows of N per tile
TILES = N // NT
KCH = D // 128             # d-chunks (contraction chunks of 128)
HH = H // 128              # h halves
GP = 3                     # d-chunks routed DVE(mul) + Scalar(accum-reduce)
GP_LAST = 5                # ...on the final group (drain balance)
GMAX = 2                   # largest group
NG = 10                    # racc columns per chunk (>= groups per slide)
WARM_MMS = 34
OUTE_LAG = 2
# group sizes per slide: pairs amortize per-op fixed costs without the
# pipeline-fill bubble quads showed (e for a group needs all its
# h-blocks); two head singles fill the DVE/ACT pipeline fast
GROUP_SCHED = {0: [1, 1] + [2] * 7, SPC - 1: [2] * 7 + [1, 1]}
GROUP_DEFAULT = [2] * 8

_NC_CACHE = {}


def _build_nc():
    bf = mybir.dt.bfloat16
    f32 = mybir.dt.float32
    AF = mybir.ActivationFunctionType
    OP = mybir.AluOpType

    nc = bacc.Bacc("TRN2", num_devices=NCORES)
    # Host-swizzled transposed layout, tile-major: per tile one contiguous
    # 1 MiB region.  xt[s, t, q, k*NT + j] = X[s, t*NT + j, k*128 + q]
    xt = nc.declare_dram_parameter("xt", [SPC, TILES, 128, KCH * NT], bf, isOutput=False)
    # w1t[q, k*H + h] = W1[h, k*128 + q]
    w1t = nc.declare_dram_parameter("w1t", [128, KCH * H], bf, isOutput=False)
    # w2rep[q, half*128 + c] = W2[0, half*128 + q]  (replicated along c)
    w2rep = nc.declare_dram_parameter("w2rep", [128, H], bf, isOutput=False)
    outp = nc.declare_dram_parameter("out", [SPC, 128, KCH], f32, isOutput=True)
    # e rows, summed on host for the softmax denominator (flat per slide)
    oute = nc.declare_dram_parameter("oute", [SPC, TILES * NT], bf, isOutput=True)

    with tile.TileContext(nc) as tc:
        with tc.tile_pool(name="const", bufs=1) as constp, \
             tc.tile_pool(name="xt", bufs=4) as xtp, \
             tc.tile_pool(name="tanh", bufs=3) as tp, \
             tc.tile_pool(name="e128", bufs=4) as ep, \
             tc.tile_pool(name="scr", bufs=2) as scrp, \
             tc.tile_pool(name="scra", bufs=2) as scrap, \
             tc.tile_pool(name="gprod", bufs=3) as gprodp, \
             tc.tile_pool(name="racc", bufs=2) as raccp, \
             tc.tile_pool(name="outsb", bufs=2) as outsbp, \
             tc.tile_pool(name="hps", bufs=2, space="PSUM") as hpsp, \
             tc.tile_pool(name="sps", bufs=1, space="PSUM") as spsp:

            # w1t gates every h-matmul: its first half precedes everything
            # in the HWDGE FIFO; second half rides between xt quarters.
            w1t_sb = constp.tile([128, KCH * H], bf)
            nc.sync.dma_start(w1t_sb[:, 0:KCH * H // 2], w1t[:, 0:KCH * H // 2])
            w2r_sb = constp.tile([128, H], bf)

            warm_sb = constp.tile([128, 256], bf)
            # memset on DVE: its preamble ends earliest, warm MMs start ASAP
            nc.vector.memset(warm_sb[:], 0.0)
            warm_ps = hpsp.tile([128, 256], f32, tag="hps")
            for _ in range(WARM_MMS):
                nc.tensor.matmul(
                    warm_ps[:, 0:128], warm_sb[:, 0:128], warm_sb[:, 0:128],
                    start=True, stop=True, skip_group_check=True,
                )

            state = {}          # per-slide persistent accumulator tiles
            pend_oute = []      # [(s, t0, G, e_g)] stores, emitted late

            def h_mms(xt_g, u, h_ps, mid=None):
                # k-outer so each arriving xt piece unblocks its MMs; `mid`
                # emits the previous tile's score MMs (and possibly the
                # previous group's exp) halfway through the block.
                for k in range(KCH):
                    if k == KCH // 2 and mid is not None:
                        mid()
                    for half in range(HH):
                        nc.tensor.matmul(
                            h_ps[:, half * NT:(half + 1) * NT],
                            w1t_sb[:, k * H + half * 128: k * H + half * 128 + 128],
                            xt_g[:, (u * KCH + k) * NT:(u * KCH + k + 1) * NT],
                            start=(k == 0), stop=(k == KCH - 1),
                        )

            def score_mms(s_ps, u, tanh_sb):
                for half in range(HH):
                    nc.tensor.matmul(
                        s_ps[:, u * NT:(u + 1) * NT],
                        w2r_sb[:, half * 128:(half + 1) * 128],
                        tanh_sb[:, half * NT:(half + 1) * NT],
                        start=(half == 0), stop=(half == HH - 1),
                        skip_group_check=True,
                    )

            def emit_exp(s, t0, G, gi, s_ps, xt_g):
                e_g = ep.tile([128, GMAX * NT], bf, tag="e")
                nc.scalar.activation(e_g[:, 0:G * NT], s_ps[:, 0:G * NT], AF.Exp)
                pend_oute.append((s, t0, G, e_g))
                return (s, gi, G, xt_g, e_g, t0)

            def emit_wsum(s, gi, G, xt_g, e_g, t0):
                racc_a, racc_d = state[s]
                last = t0 + G == TILES and s == SPC - 1
                gp = GP_LAST if last else GP
                xt4 = xt_g[:, 0:G * KCH * NT].rearrange(
                    "q (u k j) -> q u k j", u=G, k=KCH
                )
                e3 = e_g[:, 0:G * NT].rearrange("q (u j) -> q u j", u=G)
                # products for the ACT-owned chunks first (2x-rate, one op)
                # so the Scalar engine's accum copies unblock early
                prod = gprodp.tile(
                    [128, max(GMAX * GP, 2 * GP_LAST) * NT], bf, tag="prod",
                )
                prod4 = prod[:, 0:G * gp * NT].rearrange(
                    "q (u k j) -> q u k j", u=G, k=gp
                )
                if last:
                    # drain: per-chunk muls so the Scalar engine's first
                    # accum copy starts after ~0.6us instead of the full mul
                    for k in range(gp):
                        nc.vector.tensor_mul(
                            prod4[:, :, k:k + 1, :],
                            xt4[:, :, k:k + 1, :],
                            e3[:, :, None, :].broadcast_to([128, G, 1, NT]),
                        )
                else:
                    nc.vector.tensor_mul(
                        prod4[:, :, 0:gp, :],
                        xt4[:, :, 0:gp, :],
                        e3[:, :, None, :].broadcast_to([128, G, gp, NT]),
                    )
                scr = scrp.tile([128, GMAX * NT], bf, tag="scr")
                for k in range(gp, KCH):
                    nc.vector.scalar_tensor_tensor(
                        scr[:, 0:G * NT].rearrange("q (u j) -> q u j", u=G),
                        xt4[:, :, k, :],
                        1.0,
                        e3[:],
                        op0=OP.mult,
                        op1=OP.mult,
                        accum_out=racc_d[:, (k - GP) * NG + gi:
                                         (k - GP) * NG + gi + 1],
                    )
                return (s, gi, G, gp, prod4, t0)

            def emit_accums(s, gi, G, gp, prod4, t0):
                racc_a, racc_d = state[s]
                scr_a = scrap.tile([128, GMAX * NT], bf, tag="scra")
                for k in range(gp):
                    # the drain group's extra ACT chunks land in racc_d's
                    # column for that (chunk, group) — zeros elsewhere
                    acc = (
                        racc_a[:, k * NG + gi: k * NG + gi + 1]
                        if k < GP
                        else racc_d[:, (k - GP) * NG + gi:
                                    (k - GP) * NG + gi + 1]
                    )
                    nc.scalar.activation(
                        scr_a[:, 0:G * NT].rearrange("q (u j) -> q u j", u=G),
                        prod4[:, :, k, :],
                        AF.Copy,
                        accum_out=acc,
                    )
                if t0 + G == TILES:
                    out_sb = outsbp.tile([128, KCH], f32)
                    nc.vector.reduce_sum(
                        out_sb[:, 0:GP],
                        racc_a[:].rearrange("q (k g) -> q k g", k=GP),
                        axis=mybir.AxisListType.X,
                    )
                    nc.vector.reduce_sum(
                        out_sb[:, GP:KCH],
                        racc_d[:].rearrange("q (k g) -> q k g", k=KCH - GP),
                        axis=mybir.AxisListType.X,
                    )
                    # final slide's out rides the faster HWDGE path (sync
                    # is idle at the drain, and its ~0.6us first-byte beats
                    # SWDGE's ~1us + 2us completion); mid-run slides stay on
                    # gpsimd so their reduce-wait can't block xt loads
                    eng = nc.sync if s == SPC - 1 else nc.gpsimd
                    eng.dma_start(outp[s], out_sb[:])

            def flush_oute(keep):
                while len(pend_oute) > keep:
                    s_, t0_, G_, e_ = pend_oute.pop(0)
                    nc.sync.dma_start(
                        oute[s_:s_ + 1, t0_ * NT:(t0_ + G_) * NT],
                        e_[0:1, 0:G_ * NT],
                    )

            groups = []
            for s in range(SPC):
                t0 = 0
                for G in GROUP_SCHED.get(s, GROUP_DEFAULT):
                    groups.append((s, t0, G))
                    t0 += G
                assert t0 == TILES

            pend_score = None   # (s_ps, u, tanh) last tile of prev group
            cur_group = None    # (s, t0, G, gi, s_ps, xt_g) being scored
            first = True

            for s, t0, G in groups:
                if t0 == 0:
                    gi = 0
                    # separate accumulators for the ACT and DVE chunk sets —
                    # a shared tile would thread false cross-engine deps.
                    # Zero-filled: unused group-columns must fold as 0.
                    racc_a = raccp.tile(
                        [128, GP * NG], f32, tag=f"racca{s}", name=f"racca{s}",
                    )
                    racc_d = raccp.tile(
                        [128, (KCH - GP) * NG], f32,
                        tag=f"raccd{s}", name=f"raccd{s}",
                    )
                    nc.vector.memset(racc_a[:], 0.0)
                    nc.vector.memset(racc_d[:], 0.0)
                    state[s] = (racc_a, racc_d)
                xt_g = xtp.tile([128, GMAX * KCH * NT], bf, tag="xt")
                for u in range(G):
                    dst = xt_g[:, u * KCH * NT:(u + 1) * KCH * NT]
                    src = xt[s, t0 + u]
                    if first and u == 0:
                        # quarters interleaved with w1t's second half and
                        # w2rep: first h-MMs start after ~1.3 MiB
                        nc.sync.dma_start(dst[:, 0:2 * NT], src[:, 0:2 * NT])
                        nc.sync.dma_start(
                            w1t_sb[:, KCH * H // 2:], w1t[:, KCH * H // 2:],
                        )
                        for q in range(1, 4):
                            nc.sync.dma_start(
                                dst[:, q * 2 * NT:(q + 1) * 2 * NT],
                                src[:, q * 2 * NT:(q + 1) * 2 * NT],
                            )
                            if q == 1:
                                nc.sync.dma_start(w2r_sb[:], w2rep[:, :])
                    else:
                        nc.sync.dma_start(dst, src)

                s_ps = spsp.tile([128, GMAX * NT], f32, tag="sps")
                exh = [None]

                def mid0():
                    # previous group's last score MMs + its exp: exp
                    # precedes this group's tanhs in the ACT FIFO so DVE's
                    # wsum input is ready ~2us into the group
                    if pend_score is not None:
                        ps_, pu_, ptanh = pend_score
                        score_mms(ps_, pu_, ptanh)
                        exh[0] = emit_exp(*cur_group)

                tanh_prev = None
                for u in range(G):
                    h_ps = hpsp.tile(
                        [128, HH * NT], f32, name=f"h_ps{u}", tag="hps",
                    )
                    if u == 0:
                        h_mms(xt_g, 0, h_ps, mid=mid0)
                    else:
                        tp_prev = tanh_prev
                        uu = u - 1
                        h_mms(
                            xt_g, u, h_ps,
                            mid=lambda: score_mms(s_ps, uu, tp_prev),
                        )
                    tanh_u = tp.tile([128, HH * NT], bf, tag="tanh")
                    t_glob = t0 + u
                    if (s == 0 and t_glob < 2) or (
                        s == SPC - 1 and t_glob == TILES - 1
                    ):
                        # head singles + the drain tile: tanh in halves so
                        # the immediately-following score MMs (and exp)
                        # start half a tanh earlier
                        for hh in range(HH):
                            nc.scalar.activation(
                                tanh_u[:, hh * NT:(hh + 1) * NT],
                                h_ps[:, hh * NT:(hh + 1) * NT],
                                AF.Tanh,
                            )
                    else:
                        nc.scalar.activation(tanh_u[:], h_ps[:], AF.Tanh)
                    tanh_prev = tanh_u
                    if u == min(1, G - 1) and exh[0] is not None:
                        # wsum + accums for the previous group; accum
                        # copies sit after this group's early tanhs in the
                        # ACT FIFO so the PE score path never waits
                        wa = emit_wsum(*exh[0])
                        emit_accums(*wa)
                        exh[0] = None

                if G == 1 and s == 0 and t0 < 2:
                    # head singles: PE is DMA-paced here, so their score
                    # MMs ride in its stall gaps — score+exp+wsum inline,
                    # which starts DVE ~8us earlier than mid-next placement
                    score_mms(s_ps, 0, tanh_prev)
                    ex0 = emit_exp(s, t0, G, gi, s_ps, xt_g)
                    wa = emit_wsum(*ex0)
                    emit_accums(*wa)
                    pend_score = None
                else:
                    pend_score = (s_ps, G - 1, tanh_prev)
                    cur_group = (s, t0, G, gi, s_ps, xt_g)
                gi += 1
                first = False
                flush_oute(OUTE_LAG)

            # drain: score+exp+wsum for the final group only (oute flushed
            # first so the sync FIFO isn't blocked behind the out-store)
            ps_, pu_, ptanh = pend_score
            score_mms(ps_, pu_, ptanh)
            ex = emit_exp(*cur_group)
            flush_oute(0)
            wa = emit_wsum(*ex)
            emit_accums(*wa)

    nc.compile()
    return nc


def _get_nc():
    if "nc" not in _NC_CACHE:
        _NC_CACHE["nc"] = _build_nc()
    return _NC_CACHE["nc"]


def _prep_inputs(tiles_embeddings, W1, W2):
    X_bf = tiles_embeddings.astype(BF16)
    # xt[b, t, q, k, j] = X[b, t*NT + j, k*128 + q]
    xt_sw = np.ascontiguousarray(
        X_bf.reshape(B, TILES, NT, KCH, 128).transpose(0, 1, 4, 3, 2)
    ).reshape(B, TILES, 128, KCH * NT)
    # w1t[q, k, h] = W1[h, k*128 + q]
    w1t = np.ascontiguousarray(
        W1.astype(BF16).reshape(H, KCH, 128).transpose(2, 1, 0)
    ).reshape(128, KCH * H)
    # w2rep[q, half*128 + c] = W2[0, half*128 + q]
    w2rep = np.ascontiguousarray(
        np.broadcast_to(
            W2.astype(BF16).reshape(HH, 128).transpose(1, 0)[:, :, None],
            (128, HH, 128),
        )
    ).reshape(128, H)
    return [
        {
            "xt": xt_sw[c * SPC:(c + 1) * SPC],
            "w1t": w1t,
            "w2rep": w2rep,
        }
        for c in range(NCORES)
    ]


def _run(tiles_embeddings, W1, W2, **spmd_kwargs):
    nc = _get_nc()
    in_maps = _prep_inputs(tiles_embeddings, W1, W2)
    res = run_bass_kernel_spmd(nc, in_maps, core_ids=list(range(NCORES)), **spmd_kwargs)
    acc = np.concatenate([r["out"] for r in res.results], axis=0)       # [B, 128, KCH]
    e = np.concatenate([r["oute"] for r in res.results], axis=0)        # [B, TILES*NT]
    l = e.astype(np.float64).sum(axis=1)                                # [B]
    # out[b, k*128 + q] = acc[b, q, k]
    out = acc.transpose(0, 2, 1).reshape(B, D) / l[:, None]
    return out.astype(np.float32, copy=False), res


def kernel(tiles_embeddings, W1, W2):
    out, _ = _run(
        np.asarray(tiles_embeddings), np.asarray(W1), np.asarray(W2)
    )
    return out
